# revision 29
# baseline (speedup 1.0000x reference)
"""Trainium2 Bass kernel for an 8-head self-attention block with relative
position embeddings (LayerNorm -> qkv -> rel-pos attention -> out proj).

Sharding: pure data-parallel over the batch dim. B == 8 == n_cores, so each
NeuronCore processes one batch element end-to-end; no collectives.

Math notes (per batch element, per head h):
  scores = ((q+u)@k^T + (q+v)@pos^T) / 8
         = ((q+u) @ (k+pos)^T + (v-u).pos[m]) / 8
Host precomputes pos = rel_pos_emb @ w_pos^T (input-dependent only through
the weight w_pos), so on device:
  - q' = q + u_h folds into the q PSUM->SBUF copy (per-partition add),
  - kp = k + pos folds into the k PSUM->SBUF copy (tensor_tensor add),
  - the remaining bias c_h[m] = (v_h-u_h).pos_h[m] is a host-precomputed
    table that the ACT engine folds into the softmax exp:
      P^T = Exp(S_psum * 0.125 + c_col).
Softmax denominators come from a ones-column appended to V (M=65 in the
P^T @ V matmul); the division is one reciprocal_approx_fast custom-DVE op
(partition-0 SBUF source only - PSUM and partition-offset sources both
misparse on HW) + a GPSIMD partition-broadcast + one DVE multiply. The
last pair's lc1 PV chains borrow the S PSUM ring (idle after the final
exps) so all 4 end chains hold distinct banks and none WAR-stalls on a
previous chain's division. The out projection packs head pairs along K
(K=128 per matmul instead of 65), folds b_out into the PSUM->SBUF copy
(DVE broadcast add), and accumulates the last pair's contribution after
the others so it overlaps that pair's softmax divisions.

DMA: the queues are descriptor-issue-bound (~47ns per descriptor, one
descriptor per partition per contiguous chunk), so every transfer is laid
out for maximum per-partition contiguity:
  - x arrives as two [128, 2048] f32 megatiles (4 consecutive sequence
    rows per partition = one 8KB descriptor), each split across the three
    hwdge queues (sync/scalar/gpsimd). This relabels the kernel's
    sequence order: tile t=4k+j, partition p holds original row
    512k+4p+j. The relabeling is uniform across l and m, so only the
    host-side posT/cb column order and the x/output DMA patterns know
    about it - the kernel body is unchanged.
  - the small constants (posT pair-0, cb, ucol, b_out broadcast, identity)
    ride INSIDE the first x megatile as extra columns: zero extra
    descriptors for five tensors.
  - w1qk is pair-major [128, pair, kt, 256] (one 8KB descriptor per
    partition), w1v/wout2b/posT(pairs 1-3) host-pretiled to [128, ...]
    per-partition-contiguous layouts.
  - the output leaves as two [128, 4, 512] f32 megatiles (8KB
    descriptors) whose DMA pattern restores the original row order.

Everything on the matmul path runs bf16 (weights, z, q', kp, P^T, V, out
proj); LayerNorm, PSUM accumulation and the softmax bias/exp stay f32.
(fp8 P/V was tried and is ~25us faster on PE but pushes the end-to-end
error to ~3e-2, over the 2e-2 gate.)

Schedule: the per-pair scores loop is ACT-paced (2 exps of [128,1024] per
mt iteration vs 4 small score matmuls), so one contiguous "filler" chain -
a v-projection chain for pair 0, a previous pair's PV chain otherwise - is
emitted per mt iteration to keep the PE dense while ACT chews. Fillers must
be whole accumulation chains: interleaving individual chain steps between
other matmul groups measurably slows the PE stream. The PT ring holds two
pairs (bufs=4) since pair t's exps write while pair t-1's woven PV chains
still read. Score matmuls of the two heads in a pair alternate (64,128)
stationary quadrants (rows 0-63 / 64-127) so their LDWEIGHTS ping-pong.
"""

import math

import numpy as np

MODEL_DIM = 512
NUM_HEADS = 8
DIM_HEAD = 64
INNER = NUM_HEADS * DIM_HEAD
B, L = 8, 1024
EPS = 1e-5
N_CORES = 8
LT = L // 128          # l tiles
KT = MODEL_DIM // 128  # contraction (d) tiles
SCALE = DIM_HEAD ** -0.5

# xb0 f32 column layout: x | posT0(bf16 bits) | cb | ucol | boutb | ident(bits)
_XB0_X = slice(0, 2048)
_XB0_POST0 = slice(2048, 2560)   # 1024 bf16
_XB0_CB = slice(2560, 2624)      # [LT, H] f32
_XB0_UCOL = slice(2624, 2628)    # [KT] f32
_XB0_BOUT = slice(2628, 3140)    # [MODEL_DIM] f32
_XB0_IDENT = slice(3140, 3204)   # 128 bf16
XB0_COLS = 3204

_CACHE = {}


def _rel_pos_emb_np():
    # mirror reference._rel_pos_emb in float32 numpy
    rel = (np.arange(L, dtype=np.float32)[:, None]
           - np.arange(MODEL_DIM, dtype=np.float32)[None, :])
    freqs = np.exp(-np.arange(0, 2 * MODEL_DIM, 2, dtype=np.float32)
                   * np.float32(math.log(10000.0) / MODEL_DIM))
    angle = rel * freqs[None, :]
    even = (np.arange(MODEL_DIM) % 2) == 0
    return np.where(even[None, :], np.cos(angle), np.sin(angle)).astype(np.float32)


def _perm():
    """kernel sequence index l~ -> original row: l~=t*128+p, t=4k+j ->
    512k + 4p + j (partition p of x-megatile k holds 4 consecutive rows)"""
    lt = np.arange(L)
    k, j, p = lt // 512, (lt // 128) % 4, lt % 128
    return 512 * k + 4 * p + j


def _build(debug=False):
    import concourse.bacc as bacc
    import concourse.tile as tile
    from concourse import mybir

    F32 = mybir.dt.float32
    BF16 = mybir.dt.bfloat16
    AF = mybir.ActivationFunctionType
    ALU = mybir.AluOpType

    nc = bacc.Bacc('TRN2', target_bir_lowering=False)

    xb0_d = nc.dram_tensor('xb0', [128, XB0_COLS], F32, kind='ExternalInput')
    xb1_d = nc.dram_tensor('xb1', [128, 2048], F32, kind='ExternalInput')
    # pair-major qk weights: [p, pair, kt, 0:128]=q_pair, [.,128:256]=k_pair
    w1qk_d = nc.dram_tensor('w1qk', [128, KT, KT, 256], BF16, kind='ExternalInput')
    w1v_d = nc.dram_tensor('w1v', [128, KT, INNER], BF16, kind='ExternalInput')
    posT123_d = nc.dram_tensor('posT123', [128, KT - 1, L], BF16,
                               kind='ExternalInput')
    wout2b_d = nc.dram_tensor('wout2b', [128, KT, MODEL_DIM], BF16,
                              kind='ExternalInput')
    ob_d = nc.dram_tensor('ob', [L, MODEL_DIM], F32, kind='ExternalOutput')
    # output leaves as 2 megatiles: partition p of megatile k -> rows
    # 512k+4p .. 512k+4p+3 (8KB contiguous)
    ob_r = ob_d[:].rearrange('(k p r) d -> p k (r d)', p=128, r=4)
    if debug:
        dbg = {
            'dbg_w1qk': nc.dram_tensor('dbg_w1qk', [128, KT, KT, 256], BF16,
                                       kind='ExternalOutput'),
            'dbg_zT': nc.dram_tensor('dbg_zT', [128, KT, L], BF16,
                                     kind='ExternalOutput'),
            'dbg_v': nc.dram_tensor('dbg_v', [128, LT, NUM_HEADS * 65], BF16,
                                    kind='ExternalOutput'),
            'dbg_o2': nc.dram_tensor('dbg_o2', [128, KT, L], BF16,
                                     kind='ExternalOutput'),
            'dbg_pvs': nc.dram_tensor('dbg_pvs', [1, 512], F32,
                                      kind='ExternalOutput'),
            'dbg_rc': nc.dram_tensor('dbg_rc', [1, 512], F32,
                                     kind='ExternalOutput'),
            'dbg_rcb': nc.dram_tensor('dbg_rcb', [64, 512], F32,
                                      kind='ExternalOutput'),
            'dbg_qt': nc.dram_tensor('dbg_qt', [128, L], BF16,
                                     kind='ExternalOutput'),
            'dbg_kp': nc.dram_tensor('dbg_kp', [128, L], BF16,
                                     kind='ExternalOutput'),
        }

    with nc.allow_low_precision(reason="bf16 matmul pipeline"), \
            tile.TileContext(nc) as tc:
        with (
            tc.tile_pool(name='const', bufs=1) as constp,
            tc.tile_pool(name='acts', bufs=1) as acts,
            tc.tile_pool(name='wts', bufs=1) as wts,
            tc.tile_pool(name='xz', bufs=4) as xzp,
            tc.tile_pool(name='qkp', bufs=2) as qkp,
            tc.tile_pool(name='pt', bufs=4) as ptp,
            tc.tile_pool(name='rc', bufs=4) as rcp,
            tc.tile_pool(name='outp', bufs=1) as outp,
            tc.tile_pool(name='psM', bufs=2, space='PSUM') as psM,
        ):
            # ------------- megatiles / weights + the full DMA plan -------
            xm0 = wts.tile([128, XB0_COLS], F32)
            xm1 = wts.tile([128, 2048], F32)
            w1qk_sb = wts.tile([128, KT, KT, 256], BF16)
            w1v_sb = wts.tile([128, KT, INNER], BF16)
            posT_sb = wts.tile([128, KT - 1, L], BF16)
            wout2b_sb = wts.tile([128, KT, MODEL_DIM], BF16)

            # per-queue program order == service order; x first, then
            # weights by first-use time. Thirds for x (all queues), halves
            # for the big weights.
            TH = [(0, 43), (43, 86), (86, 128)]
            ENG3 = (nc.sync, nc.scalar, nc.gpsimd)
            for (a, b), eng in zip(TH, ENG3):
                eng.dma_start(xm0[a:b, :], xb0_d[a:b, :])
            for (a, b), eng in zip(TH, ENG3):
                eng.dma_start(xm1[a:b, :], xb1_d[a:b, :])
            nc.sync.dma_start(w1qk_sb[0:64], w1qk_d[0:64])
            nc.scalar.dma_start(w1qk_sb[64:128], w1qk_d[64:128])
            nc.gpsimd.dma_start(w1v_sb[0:64], w1v_d[0:64])
            nc.scalar.dma_start(w1v_sb[64:128], w1v_d[64:128])
            nc.sync.dma_start(posT_sb[0:64], posT123_d[0:64])
            nc.gpsimd.dma_start(posT_sb[64:128], posT123_d[64:128])
            nc.sync.dma_start(wout2b_sb[0:64], wout2b_d[0:64])
            nc.gpsimd.dma_start(wout2b_sb[64:128], wout2b_d[64:128])

            # constant views into xm0 (rode along with the x megatile)
            xm0b = xm0[:].bitcast(BF16)
            posT0_v = xm0b[:, 2 * _XB0_POST0.start:2 * _XB0_POST0.stop]
            cb_v = xm0[:, _XB0_CB].rearrange('p (t h) -> p t h', h=NUM_HEADS)
            ucol_v = xm0[:, _XB0_UCOL]
            boutb_v = xm0[:, _XB0_BOUT]
            ident_v = xm0b[:, 2 * _XB0_IDENT.start:2 * _XB0_IDENT.stop]

            eps_sb = constp.tile([128, 1], F32)
            nc.vector.memset(eps_sb[:], EPS)

            zT = acts.tile([128, KT, L], BF16)
            # V with a ones column per head: [m-part, mt, h*65+c]
            v_sb = acts.tile([128, LT, NUM_HEADS * 65], BF16)
            nc.vector.memset(
                v_sb[:].rearrange('p t (h c) -> p t h c', c=65)[:, :, :, 64:65], 1.0)
            # normalized attention output, head pairs packed: [128, pair, L]
            outT2 = acts.tile([128, KT, L], BF16)
            # output megatiles: [:, j, :] = out rows of l~-tile 4k+j
            otm = [outp.tile([128, KT, MODEL_DIM], F32, name=f'otm{k}')
                   for k in range(2)]

            # ---------------- LayerNorm + transpose into zT ----------------
            # software-pipelined by one stage: stats/aggr of tile lt are
            # emitted before the normalize/transpose of tile lt-1 so the
            # in-order DVE queue reaches zt(0) as soon as its rstd is ready
            ln_state = {}

            def xview(t):
                j = t % 4
                return (xm0 if t < 4 else xm1)[:, 512 * j:512 * (j + 1)]

            def ln_front(lt):
                xt = xview(lt)
                stats = xzp.tile([128, 6], F32, tag='stats')
                nc.vector.bn_stats(stats[:], xt)
                mv = xzp.tile([128, 2], F32, tag='mv')
                nc.vector.bn_aggr(mv[:], stats[:])
                lnv = xzp.tile([128, 1], F32, tag='lnv')
                nc.scalar.activation(lnv[:], mv[:, 1:2], AF.Ln, bias=eps_sb[:], scale=1.0)
                rstd = xzp.tile([128, 1], F32, tag='rstd', name=f'rstd{lt}')
                nc.scalar.activation(rstd[:], lnv[:], AF.Exp, scale=-0.5)
                ln_state[lt] = (xt, mv, rstd)

            def ln_back(lt):
                xt, mv, rstd = ln_state.pop(lt)
                nmr = xzp.tile([128, 1], F32, tag='nmr')
                nc.vector.scalar_tensor_tensor(nmr[:], mv[:, 0:1], -1.0, rstd[:],
                                               op0=ALU.mult, op1=ALU.mult)
                zt = xzp.tile([128, MODEL_DIM], BF16, tag='zt')
                nc.vector.tensor_scalar(zt[:], xt, rstd[:], nmr[:],
                                        op0=ALU.mult, op1=ALU.add)
                tp = psM.tile([128, L], F32, tag='S')
                tpb = tp.bitcast(BF16)
                for c in range(KT):
                    nc.tensor.transpose(tpb[:, c * 128:(c + 1) * 128],
                                        zt[:, c * 128:(c + 1) * 128], ident_v)
                nc.vector.tensor_copy(
                    zT[:, :, lt * 128:(lt + 1) * 128],
                    tpb[:, 0:512].rearrange('p (c l) -> p c l', c=KT))

            for lt in range(LT):
                ln_front(lt)
                if lt >= 1:
                    ln_back(lt - 1)
            ln_back(LT - 1)

            # ---------------- interleaved projections + attention ----------
            qts, kps = {}, {}

            def postT(t, ls):
                return posT0_v[:, ls] if t == 0 else posT_sb[:, t - 1, ls]

            def qk_chains(t):
                """allocate qt/kp for pair t and return the 4 projection
                chains as closures (woven into the previous pair's scores
                loop as PE fillers; they depend only on zT and w1qk)"""
                qt = qkp.tile([128, L], BF16, tag='qT', name=f'qT{t}')
                kp = qkp.tile([128, L], BF16, tag='kT', name=f'kT{t}')
                qts[t], kps[t] = qt, kp
                chains = []
                for which, dst in ((1, kp), (0, qt)):
                    for lc in range(2):
                        def chain(which=which, dst=dst, lc=lc):
                            ws = slice(which * 128, (which + 1) * 128)
                            ls = slice(lc * 512, (lc + 1) * 512)
                            acc = psM.tile([128, 512], F32, tag='qkv',
                                           name=f'qk{t}_{which}_{lc}')
                            for kt in range(KT):
                                nc.tensor.matmul(
                                    acc[:], w1qk_sb[:, t, kt, ws],
                                    zT[:, kt, ls],
                                    start=(kt == 0), stop=(kt == KT - 1))
                            if which == 0:
                                # q' = q + u_h  (per-partition add)
                                nc.vector.tensor_scalar_add(dst[:, ls], acc[:],
                                                            ucol_v[:, t:t + 1])
                            else:
                                # kp = k + pos
                                nc.vector.tensor_tensor(dst[:, ls], acc[:],
                                                        postT(t, ls), op=ALU.add)
                        chains.append(chain)
                return chains

            def emit_scores(t, fillers=()):
                """Scores + exp for heads 2t, 2t+1, interleaved at mt grain so
                the two heads' (64,128) stationary tiles ping-pong quadrants.
                The scores loop is ACT-paced (2 exps per mt vs 4 small score
                matmuls), so one contiguous filler chain (a v-projection or a
                previous pair's PV chain) is emitted per mt iteration to keep
                the PE dense while ACT chews."""
                qt, kp = qts[t], kps[t]
                fillers = list(fillers)
                PTs = {}
                for h in (2 * t, 2 * t + 1):
                    PTs[h] = ptp.tile([128, LT, L], BF16, tag='PT', name=f'PT{h}')
                for mt in range(LT):
                    for h in (2 * t, 2 * t + 1):
                        hp = 64 * (h % 2)
                        prow = slice(hp, hp + 64)
                        sacc = psM.tile([128, L], F32, tag='S')
                        for lc in range(2):
                            ls = slice(lc * 512, (lc + 1) * 512)
                            nc.tensor.matmul(sacc[:, ls],
                                             kp[prow, mt * 128:(mt + 1) * 128],
                                             qt[prow, ls], start=True, stop=True)
                        nc.scalar.activation(PTs[h][:, mt, :], sacc[:], AF.Exp,
                                             bias=cb_v[:, mt, h:h + 1], scale=SCALE)
                    if fillers:
                        fillers.pop(0)()
                while fillers:
                    fillers.pop(0)()
                return PTs

            def pv_step(pvacc, h, PT, lc, mt):
                ls = slice(lc * 512, (lc + 1) * 512)
                nc.tensor.matmul(pvacc, v_sb[:, mt, h * 65:(h + 1) * 65],
                                 PT[:, mt, ls],
                                 start=(mt == 0), stop=(mt == LT - 1))

            def pv_finish(pvacc, h, lc):
                """normalize the finished P^T @ [V|1] into packed outT2"""
                pair, hp = divmod(h, 2)
                rows = slice(hp * 64, hp * 64 + 64)
                ls = slice(lc * 512, (lc + 1) * 512)
                zrow = rcp.tile([1, 512], F32, tag='zrow')
                nc.vector.tensor_copy(zrow[:], pvacc[64:65, :])
                rc = rcp.tile([1, 512], F32, tag='rc')
                # custom-DVE op: partition-0 SBUF source only
                nc.vector.reciprocal_approx_fast(rc[:], zrow[:])
                rcb = rcp.tile([64, 512], F32, tag='rcb')
                nc.gpsimd.partition_broadcast(rcb[:], rc[:])
                nc.vector.tensor_mul(outT2[rows, pair, ls], pvacc[0:64, :], rcb[:])
                fin_state['last'] = (zrow, rc, rcb)

            fin_state = {}

            def emit_pv_block(hs, PTs_, lc):
                """contiguous PV chains for the given heads at one lc.
                lc1 borrows the S ring so all 4 end chains hold distinct
                PSUM slots and none WAR-stalls on another's division."""
                for h in hs:
                    pv_chain(h, PTs_[h], lc, end=(lc == 1))

            def pv_chain(h, PT, lc, end=False):
                if end:
                    big = psM.tile([128, L], F32, tag='S', name=f'pvE{h}_{lc}')
                    pvacc = big[0:65, 0:512]
                else:
                    pvacc = psM.tile([65, 512], F32, tag='pv',
                                     name=f'pv{h}_{lc}')[:]
                for mt in range(LT):
                    pv_step(pvacc, h, PT, lc, mt)
                pv_finish(pvacc, h, lc)

            def v_chain(mt):
                acc = psM.tile([128, 512], F32, tag='qkv', name=f'vacc{mt}')
                for kt in range(KT):
                    nc.tensor.matmul(acc[:], zT[:, kt, mt * 128:(mt + 1) * 128],
                                     w1v_sb[:, kt, :],
                                     start=(kt == 0), stop=(kt == KT - 1))
                nc.vector.tensor_copy(
                    v_sb[:, mt, :].rearrange('p (h c) -> p h c', c=65)[:, :, 0:64],
                    acc[:].rearrange('p (h c) -> p h c', c=64))

            def emit_outproj(lts):
                # the last head pair's contribution is accumulated LAST so the
                # first matmuls of each chain run while that pair's softmax
                # divisions are still in flight on DVE/Pool. b_out is folded
                # into the PSUM->SBUF copy; each finished output megatile
                # leaves as one 8KB-descriptor DMA split across idle queues.
                for lt in lts:
                    k, j = divmod(lt, 4)
                    facc = psM.tile([128, MODEL_DIM], F32, tag='qkv')
                    for p in range(KT - 1):
                        nc.tensor.matmul(facc[:], outT2[:, p, lt * 128:(lt + 1) * 128],
                                         wout2b_sb[:, p, :],
                                         start=(p == 0), stop=False)
                    nc.tensor.matmul(facc[:], outT2[:, KT - 1, lt * 128:(lt + 1) * 128],
                                     wout2b_sb[:, KT - 1, :],
                                     start=False, stop=True)
                    nc.vector.tensor_tensor(otm[k][:, j, :], facc[:], boutb_v,
                                            op=ALU.add)
                    if lt == 3:
                        nc.sync.dma_start(ob_r[0:64, 0, :], otm[0][0:64])
                        nc.gpsimd.dma_start(ob_r[64:128, 0, :], otm[0][64:128])
                    elif lt == 7:
                        for (a, b), eng in zip(TH, ENG3):
                            eng.dma_start(ob_r[a:b, 1, :], otm[1][a:b])

            prev = None
            for c in qk_chains(0):
                c()
            for t in range(KT):
                if prev is None:
                    base = [(lambda mt=mt: v_chain(mt)) for mt in range(LT)]
                else:
                    base = [(lambda h=h, lc=lc, PT=prev[h]: pv_chain(h, PT, lc))
                            for lc in (0, 1) for h in prev.keys()]
                nxt = qk_chains(t + 1) if t + 1 < KT else []
                # interleave so the next pair's projections (and their DVE
                # folds) finish well before its scores loop begins
                fillers = []
                for i in range(max(len(base), len(nxt))):
                    if i < len(base):
                        fillers.append(base[i])
                    if i < len(nxt):
                        fillers.append(nxt[i])
                prev = emit_scores(t, fillers)
            # last pair: all PV chains; the lc0 divisions (DVE/Pool latency)
            # hide under the lc1 chains, and the lc1 divisions hide under the
            # first half of the output projection
            phs = list(prev.keys())
            emit_pv_block(phs, prev, 0)
            emit_pv_block(phs, prev, 1)
            emit_outproj(range(0, 4))
            emit_outproj(range(4, 8))

            if debug:
                zrow, rc, rcb = fin_state['last']
                nc.sync.dma_start(dbg['dbg_w1qk'][:], w1qk_sb[:])
                nc.sync.dma_start(dbg['dbg_zT'][:], zT[:])
                nc.sync.dma_start(dbg['dbg_v'][:], v_sb[:])
                nc.sync.dma_start(dbg['dbg_o2'][:], outT2[:])
                nc.sync.dma_start(dbg['dbg_pvs'][:], zrow[:])
                nc.sync.dma_start(dbg['dbg_rc'][:], rc[:])
                nc.sync.dma_start(dbg['dbg_rcb'][:], rcb[:])
                nc.sync.dma_start(dbg['dbg_qt'][:], qts[KT - 1][:])
                nc.sync.dma_start(dbg['dbg_kp'][:], kps[KT - 1][:])

    # Force all activations (Ln/Exp/Identity) onto the single table set that
    # contains them all — otherwise the table-load picker alternates between
    # the natural_log and exp sets, paying a ~2.7us table load per switch.
    import concourse.bacc as bacc_mod
    orig_tables = bacc_mod.get_activation_tables

    def _only_ln_exp(arch):
        t = orig_tables(arch)
        return {name: (funcs if name == 'natural_log_exp_and_others' else
                       type(funcs)())
                for name, funcs in t.items()}

    bacc_mod.get_activation_tables = _only_ln_exp
    try:
        nc.compile()
    finally:
        bacc_mod.get_activation_tables = orig_tables
    return nc


def _host_prep(x, gamma, beta, w_qkv, b_qkv, w_pos, w_out, b_out, u_bias, v_bias):
    """Host-side layout prep. Returns (common_inputs, per_core_input_list)."""
    import ml_dtypes
    BF = ml_dtypes.bfloat16
    W1 = (gamma[:, None] * w_qkv.T).astype(np.float32)        # [D, 3*INNER]
    b1 = (b_qkv + beta @ w_qkv.T).astype(np.float32)
    if np.any(b1 != 0):
        raise NotImplementedError("nonzero qkv bias not supported by this kernel")
    # pair-major qk weights: [p, pair, kt, 0:128]=q_pair | [..,128:256]=k_pair
    wqk = W1[:, :2 * INNER].reshape(MODEL_DIM, 2, KT, 128)     # [D, q/k, pair, 128]
    wqk = wqk.transpose(0, 2, 1, 3).reshape(MODEL_DIM, KT, 256)
    w1qk = np.ascontiguousarray(
        wqk.reshape(KT, 128, KT, 256).transpose(1, 2, 0, 3)).astype(BF)
    w1v = np.ascontiguousarray(
        W1[:, 2 * INNER:].reshape(KT, 128, INNER).transpose(1, 0, 2)).astype(BF)
    perm = _perm()
    # pos = rel_pos_emb @ w_pos^T, computed on host: [L(m), INNER];
    # columns of posT / rows of cb are in kernel (permuted) sequence order
    pos = (_rel_pos_emb_np() @ w_pos.T).astype(np.float32)
    posT = np.ascontiguousarray(pos.T[:, perm]).astype(BF)     # [INNER, L~]
    posT_t = np.ascontiguousarray(
        posT.reshape(KT, 128, L).transpose(1, 0, 2))           # [128, pair, L~]
    # exp bias table c_h[m] = SCALE * (v_h - u_h) . pos_h[m]
    dvu = (v_bias - u_bias).astype(np.float32)                 # [H, Dh]
    cb = np.einsum('mhd,hd->mh', pos.reshape(L, NUM_HEADS, DIM_HEAD), dvu)
    cb = (cb * SCALE)[perm]                                    # [L~, H]
    cb = np.ascontiguousarray(
        cb.reshape(LT, 128, NUM_HEADS).transpose(1, 0, 2))     # [128, LT, H]
    # u columns: ucol[:, t] = [u_{2t} | u_{2t+1}]
    ucol = np.ascontiguousarray(
        u_bias.reshape(KT, 128).T).astype(np.float32)          # [128, KT]
    # out projection, head pairs packed along K
    wout2b = np.ascontiguousarray(
        w_out.T.reshape(KT, 128, MODEL_DIM).transpose(1, 0, 2)).astype(BF)
    boutb = np.broadcast_to(b_out.astype(np.float32), (128, MODEL_DIM))
    ident = np.eye(128, dtype=np.float32).astype(BF)

    # xb0 trailer (common across cores): posT0 bits | cb | ucol | bout | ident
    trailer = np.concatenate([
        np.ascontiguousarray(posT_t[:, 0, :]).view(np.float32),
        cb.reshape(128, LT * NUM_HEADS),
        ucol,
        boutb,
        np.ascontiguousarray(ident).view(np.float32),
    ], axis=1)
    assert trailer.shape == (128, XB0_COLS - 2048), trailer.shape

    common = {'w1qk': w1qk, 'w1v': w1v, 'posT123': posT_t[:, 1:, :].copy(),
              'wout2b': wout2b}
    xs = []
    for b in range(N_CORES):
        xb = np.ascontiguousarray(x[b], np.float32)
        xs.append({
            'xb0': np.ascontiguousarray(
                np.concatenate([xb[0:512].reshape(128, 2048), trailer], axis=1)),
            'xb1': np.ascontiguousarray(xb[512:1024].reshape(128, 2048)),
        })
    return common, xs


def kernel(x, gamma, beta, w_qkv, b_qkv, w_pos, w_out, b_out, u_bias, v_bias):
    x = np.asarray(x, np.float32)
    args = [np.asarray(a, np.float32) for a in
            (gamma, beta, w_qkv, b_qkv, w_pos, w_out, b_out, u_bias, v_bias)]
    common, xs = _host_prep(x, *args)

    if 'nc' not in _CACHE:
        _CACHE['nc'] = _build()
    nc = _CACHE['nc']

    from concourse.bass_utils import run_bass_kernel_spmd
    in_maps = [{**xs[b], **common} for b in range(N_CORES)]
    res = run_bass_kernel_spmd(nc, in_maps, core_ids=list(range(N_CORES)))
    return np.stack([res.results[b]['ob'] for b in range(N_CORES)], axis=0)
